# revision 1
# baseline (speedup 1.0000x reference)
"""Trainium2 Bass/Tile kernel: symmetric contrastive loss (CLIP-style).

Distribution: data-parallel over B across 8 NeuronCores.  Each core MLPs +
l2-normalizes its 2048-row shard of both branches, AllGathers the normalized
num-projections (bf16, 512KB/rank), computes its row-block of the 16384^2
logit matrix tile-by-tile (never materialized), and reduces:

  * rows  (i2n): ACT Exp with fused accum_out -> per-row sum(exp) locally
  * cols  (n2i): ones-matmul partition sums accumulated in PSUM, then one
    AllReduce-add of [colsum(16384) | sum(lse_rows) - sum(diag) | sum(diag)]

Logits are bounded (|cos|/temp <= 10) so logsumexp needs no max shift; plain
fp32 exp-sums are exact enough.  Temperature is folded into the projections
via scale 1/sqrt(temp) so no runtime scalar is needed inside Exp; the l2
normalization itself is exp(-0.5*ln(|z|^2) - 0.5*log_temp) on ACT (Rsqrt on
ACT is banned for accuracy).
"""

import numpy as np

N_CORES = 8
B = 16384
D_IMG = 2048
D_NUM = 256
P = 128

_NC_CACHE = {}


def build(b_total=B, d_img=D_IMG, d_num=D_NUM, n_cores=N_CORES):
    """Build + compile the Bass module. Returns the compiled Bacc object."""
    key = (b_total, d_img, d_num, n_cores)
    if key in _NC_CACHE:
        return _NC_CACHE[key]

    import concourse.bacc as bacc
    import concourse.bass as bass
    import concourse.mybir as mybir
    import concourse.tile as tile

    dt = mybir.dt
    AF = mybir.ActivationFunctionType
    Alu = mybir.AluOpType
    AX = mybir.AxisListType
    f32 = dt.float32
    bf16 = dt.bfloat16

    BL = b_total // n_cores          # local rows per core
    assert BL % 512 == 0 and b_total % 1024 == 0
    NRT = BL // 512                  # 512-wide row tiles (MLP / transpose)
    NRC = BL // 128                  # 128-row chunks (main pass)
    KI = d_img // 128                # contraction tiles, img MLP1
    KN = d_num // 128
    CW = 1024                        # main-pass column supertile width
    NCT = b_total // CW
    NH = CW // 512
    ARW = b_total + 64               # AllReduce payload width

    nc = bacc.Bacc("TRN2", target_bir_lowering=False, debug=False,
                   num_devices=n_cores)

    img = nc.dram_tensor("img_feat", [BL, d_img], f32, kind="ExternalInput").ap()
    num = nc.dram_tensor("num_feat", [BL, d_num], f32, kind="ExternalInput").ap()
    Wi1 = nc.dram_tensor("Wi1", [d_img, P], f32, kind="ExternalInput").ap()
    bi1 = nc.dram_tensor("bi1", [P, 1], f32, kind="ExternalInput").ap()
    Wi2 = nc.dram_tensor("Wi2", [P, P], f32, kind="ExternalInput").ap()
    bi2 = nc.dram_tensor("bi2", [P, 1], f32, kind="ExternalInput").ap()
    Wn1 = nc.dram_tensor("Wn1", [d_num, P], f32, kind="ExternalInput").ap()
    bn1 = nc.dram_tensor("bn1", [P, 1], f32, kind="ExternalInput").ap()
    Wn2 = nc.dram_tensor("Wn2", [P, P], f32, kind="ExternalInput").ap()
    bn2 = nc.dram_tensor("bn2", [P, 1], f32, kind="ExternalInput").ap()
    ltm = nc.dram_tensor("log_temp", [1, 1], f32, kind="ExternalInput").ap()
    loss = nc.dram_tensor("loss", [1, 1], f32, kind="ExternalOutput").ap()

    rg = [list(range(n_cores))]

    with tile.TileContext(nc) as tc:
        with (
            tc.tile_pool(name="sb", bufs=1) as sb,
            tc.tile_pool(name="stream", bufs=3) as st,
            tc.tile_pool(name="vstage", bufs=2) as vs,
            tc.tile_pool(name="xtp", bufs=2) as xtp,
            tc.tile_pool(name="xsp", bufs=2) as xsp,
            tc.tile_pool(name="dram", bufs=1, space="DRAM") as dram,
        ):
            # ---------------- constants ----------------
            ones_kb = sb.tile([P, 1], bf16)
            nc.vector.memset(ones_kb[:], 1.0)
            ones_kf = sb.tile([P, 1], f32)
            nc.vector.memset(ones_kf[:], 1.0)
            ones_1f = sb.tile([1, P], f32)
            nc.vector.memset(ones_1f[:], 1.0)
            zpad = sb.tile([1, 64], f32)
            nc.vector.memset(zpad[:], 0.0)
            # identity (bf16) for PE-mode transposes: (free_idx - part_idx)==0
            idn_i = sb.tile([P, P], dt.int32)
            nc.gpsimd.iota(idn_i[:], pattern=[[1, P]], base=0,
                           channel_multiplier=-1)
            idn = sb.tile([P, P], bf16)
            nc.vector.tensor_scalar(idn[:], idn_i[:], 0, None,
                                    op0=Alu.is_equal)
            idn_f = sb.tile([P, P], f32)
            nc.vector.tensor_scalar(idn_f[:], idn_i[:], 0, None,
                                    op0=Alu.is_equal)

            # num input first -- it gates the whole AllGather chain.
            xs_n = sb.tile([P, NRC, d_num], f32)
            nc.sync.dma_start(xs_n[:], num.rearrange("(g p) e -> p g e", p=P))

            # num-branch weights next: loaded via HWDGE (sync) as fp32 + DVE
            # cast so they never queue behind the big img SWDGE cast-DMAs.
            wn1_f = sb.tile([P, KN * P], f32)
            nc.sync.dma_start(wn1_f.rearrange("p (k m) -> p k m", k=KN),
                              Wn1.rearrange("(k p) m -> p k m", p=P))
            wn1_sb = sb.tile([P, KN * P], bf16)
            nc.vector.tensor_copy(wn1_sb[:], wn1_f[:])
            wn2_f = sb.tile([P, P], f32)
            nc.sync.dma_start(wn2_f[:], Wn2)
            wn2_sb = sb.tile([P, P], bf16)
            nc.vector.tensor_copy(wn2_sb[:], wn2_f[:])
            bn1_sb = sb.tile([P, 1], f32)
            nc.sync.dma_start(bn1_sb[:], bn1)
            bn2_sb = sb.tile([P, 1], f32)
            nc.sync.dma_start(bn2_sb[:], bn2)
            lt_sb = sb.tile([1, 1], f32)
            nc.sync.dma_start(lt_sb[:], ltm)
            nhlt = sb.tile([1, 1], f32)        # -0.5 * log_temp
            nc.vector.tensor_scalar_mul(nhlt[:], lt_sb[:], -0.5)

            # ---------------- DRAM scratch ----------------
            # AllGather split in two halves so the main pass can start after
            # the first half lands; AllReduce split so half overlaps compute.
            BH = BL // 2
            ag_in_a = dram.tile([P, BH], bf16)
            ag_in_b = dram.tile([P, BH], bf16)
            ag_out_a = dram.tile([n_cores * P, BH], bf16, addr_space="Shared")
            ag_out_b = dram.tile([n_cores * P, BH], bf16, addr_space="Shared")
            ARH = b_total // 2
            ar_in = dram.tile([1, ARW], f32)
            ar_out_a = dram.tile([1, ARH], f32, addr_space="Shared")
            ar_out_b = dram.tile([1, ARW - ARH], f32, addr_space="Shared")

            # ---------------- persistent SBUF ----------------
            xnT = sb.tile([P, KN * BL], bf16)   # num input, transposed
            h1n = sb.tile([P, BL], bf16)
            h1i = sb.tile([P, BL], bf16)
            zn = sb.tile([P, BL], bf16)
            zi = sb.tile([P, BL], bf16)
            ntl = sb.tile([P, BL], bf16)        # normalized num proj (local)
            itl = sb.tile([P, BL], bf16)        # normalized img proj (local)
            npf = sb.tile([P, b_total], bf16)   # gathered num proj (all cores)
            rowacc = sb.tile([P, NRC * NCT], f32)
            dsum = sb.tile([1, 1], f32)         # running sum of diag
            nc.vector.memset(dsum[:], 0.0)

            def mlp2_norm(pp, h1, w2, b2, z, outp):
                """z = w2.T@h1 + b2 (transposed layout); outp = z * inv, with
                inv[i] = exp(-0.5*ln(|z_i|^2) - 0.5*log_temp).  Per-row-tile
                so outp slices become ready incrementally (prologue latency
                matters more than the extra ACT table switches, which land in
                otherwise-idle ACT time)."""
                for rt in range(NRT):
                    sl = slice(rt * 512, (rt + 1) * 512)
                    pz = pp.tile([P, 512], f32, tag="zb", name="pz")
                    nc.tensor.matmul(pz[:], w2[:], h1[:, sl])
                    nc.scalar.activation(z[:, sl], pz[:], AF.Identity, bias=b2[:])
                    sq = st.tile([P, 512], bf16, tag="sq", name="sq")
                    nc.scalar.activation(sq[:], pz[:], AF.Square, bias=b2[:])
                    pv = pp.tile([P, 512], f32, tag="v", name="pv")
                    nc.tensor.matmul(pv[:1, :], ones_kb[:], sq[:])
                    lnv = vs.tile([1, 512], f32, tag="lnv", name="lnv")
                    nc.scalar.activation(lnv[:], pv[:1, :], AF.Ln)
                    inv = vs.tile([1, 512], f32, tag="inv", name="inv")
                    nc.scalar.activation(inv[:], lnv[:], AF.Exp,
                                         bias=nhlt[:], scale=-0.5)
                    pb = pp.tile([P, 512], f32, tag="zb", name="pb")
                    nc.tensor.matmul(pb[:], ones_1f[:], inv[:])
                    nc.vector.tensor_mul(outp[:, sl], z[:, sl], pb[:])

            # ---------------- num branch + AllGather ----------------
            with tc.tile_pool(name="pp1", bufs=2, space="PSUM") as pp:
                # f32 PE transposes + cast copy from the early xs_n load
                for dk in range(KN):
                    for gb in range(NRC // 4):
                        pt = pp.tile([P, 512], f32, tag="pt", name="ptn")
                        for q in range(4):
                            nc.tensor.transpose(
                                pt[:, q * P:(q + 1) * P],
                                xs_n[:, gb * 4 + q, dk * P:(dk + 1) * P],
                                idn_f[:])
                        nc.vector.tensor_copy(
                            xnT[:, dk * BL + gb * 512: dk * BL + gb * 512 + 512],
                            pt[:])
                for rt in range(NRT):
                    sl = slice(rt * 512, (rt + 1) * 512)
                    ph = pp.tile([P, 512], f32, tag="h", name="ph")
                    for k in range(KN):
                        nc.tensor.matmul(
                            ph[:], wn1_sb[:, k * P:(k + 1) * P],
                            xnT[:, k * BL + rt * 512: k * BL + rt * 512 + 512],
                            start=(k == 0), stop=(k == KN - 1))
                    nc.scalar.activation(h1n[:, sl], ph[:], AF.Relu, bias=bn1_sb[:])
                mlp2_norm(pp, h1n, wn2_sb, bn2_sb, zn, ntl)

            npf_v = npf.rearrange("p (r c) -> p r c", c=BL)
            nc.sync.dma_start(ag_in_a[:], ntl[:, 0:BH])
            nc.gpsimd.collective_compute(
                "AllGather", Alu.bypass, replica_groups=rg,
                ins=[ag_in_a.opt()], outs=[ag_out_a.opt()])
            nc.sync.dma_start(ag_in_b[:], ntl[:, BH:BL])
            nc.gpsimd.collective_compute(
                "AllGather", Alu.bypass, replica_groups=rg,
                ins=[ag_in_b.opt()], outs=[ag_out_b.opt()])
            nc.sync.dma_start(npf_v[:, :, 0:BH],
                              ag_out_a.rearrange("(r p) n -> p r n", p=P))
            nc.sync.dma_start(npf_v[:, :, BH:BL],
                              ag_out_b.rearrange("(r p) n -> p r n", p=P))

            # img-branch weights (after AG trigger; not on its critical path)
            wi1_sb = sb.tile([P, KI * P], bf16)
            nc.gpsimd.dma_start(wi1_sb.rearrange("p (k m) -> p k m", k=KI),
                                Wi1.rearrange("(k p) m -> p k m", p=P))
            wi2_sb = sb.tile([P, P], bf16)
            nc.gpsimd.dma_start(wi2_sb[:], Wi2)
            bi1_sb = sb.tile([P, 1], f32)
            nc.sync.dma_start(bi1_sb[:], bi1)
            bi2_sb = sb.tile([P, 1], f32)
            nc.sync.dma_start(bi2_sb[:], bi2)

            # ---------------- img branch ----------------
            with tc.tile_pool(name="pp2", bufs=2, space="PSUM") as pp:
                for rb in range(NRT):
                    rsl = slice(rb * 512, (rb + 1) * 512)
                    xs = xsp.tile([P, 4, d_img], bf16, tag="xsi", name="xsi")
                    nc.gpsimd.dma_start(
                        xs[:], img[rsl, :].rearrange("(q p) e -> p q e", p=P))
                    xtb = xtp.tile([P, KI * 512], bf16, tag="xt", name="xtb")
                    for dk in range(KI):
                        pt = pp.tile([P, 512], bf16, tag="pt", name="pt")
                        for q in range(4):
                            nc.tensor.transpose(
                                pt[:, q * P:(q + 1) * P],
                                xs[:, q, dk * P:(dk + 1) * P], idn[:])
                        nc.vector.tensor_copy(
                            xtb[:, dk * 512:(dk + 1) * 512], pt[:])
                    ph = pp.tile([P, 512], f32, tag="h", name="phi")
                    for k in range(KI):
                        nc.tensor.matmul(
                            ph[:], wi1_sb[:, k * P:(k + 1) * P],
                            xtb[:, k * 512:(k + 1) * 512],
                            start=(k == 0), stop=(k == KI - 1))
                    nc.scalar.activation(h1i[:, rsl], ph[:], AF.Relu,
                                         bias=bi1_sb[:])
                mlp2_norm(pp, h1i, wi2_sb, bi2_sb, zi, itl)
                # diagonal: l_ii = sum_p itl[p,i] * ntl[p,i]; accumulate sum
                for rt in range(NRT):
                    sl = slice(rt * 512, (rt + 1) * 512)
                    prod = st.tile([P, 512], bf16, tag="sq", name="prod")
                    nc.vector.tensor_mul(prod[:], itl[:, sl], ntl[:, sl])
                    pd = pp.tile([P, 512], f32, tag="v", name="pd")
                    nc.tensor.matmul(pd[:1, :], ones_kb[:], prod[:])
                    dred = vs.tile([1, 1], f32, tag="dred", name="dred")
                    nc.vector.reduce_sum(dred[:], pd[:1, :], axis=AX.X)
                    nc.vector.tensor_add(dsum[:], dsum[:], dred[:])

            # ---------------- main pass ----------------
            # ct processing order: supertiles fully covered by AG half a
            # first, so the main pass starts before AG half b completes.
            ct_a = [ct for ct in range(NCT)
                    if (ct * CW) % BL + CW <= BH]
            ct_order = ct_a + [ct for ct in range(NCT) if ct not in ct_a]
            NHALF = NCT // 2
            with (
                tc.tile_pool(name="pl", bufs=3, space="PSUM") as plp,
                tc.tile_pool(name="pc", bufs=1, space="PSUM") as pcp,
            ):
                for pos, ct in enumerate(ct_order):
                    pcol = pcp.tile([P, CW], f32, tag="pc", name="pcol")
                    for rc in range(NRC):
                        plog = plp.tile([P, CW], f32, tag="pl", name="plog")
                        for h in range(NH):
                            nc.tensor.matmul(
                                plog[:, h * 512:(h + 1) * 512],
                                itl[:, rc * P:(rc + 1) * P],
                                npf[:, ct * CW + h * 512: ct * CW + (h + 1) * 512])
                        e = st.tile([P, CW], bf16, tag="e", name="e", bufs=4)
                        slot = rc * NCT + ct
                        nc.scalar.activation(e[:], plog[:], AF.Exp,
                                             accum_out=rowacc[:, slot:slot + 1])
                        for h in range(NH):
                            nc.tensor.matmul(
                                pcol[:1, h * 512:(h + 1) * 512],
                                ones_kb[:], e[:, h * 512:(h + 1) * 512],
                                start=(rc == 0), stop=(rc == NRC - 1))
                    cst = vs.tile([1, CW], f32, tag="cst", name="cst")
                    nc.vector.tensor_copy(cst[:], pcol[:1, :])
                    nc.sync.dma_start(ar_in[:1, pos * CW:(pos + 1) * CW], cst[:])
                    if pos == NHALF - 1:
                        # first half of colsums complete -> overlap AllReduce
                        nc.gpsimd.collective_compute(
                            "AllReduce", Alu.add, replica_groups=rg,
                            ins=[ar_in[:1, 0:ARH].opt()], outs=[ar_out_a.opt()])

                # ---- row direction partials ----
                rowsum = sb.tile([P, NRC], f32)
                nc.vector.reduce_sum(
                    rowsum[:],
                    rowacc.rearrange("p (rc ct) -> p rc ct", ct=NCT), axis=AX.X)
                lse_r = sb.tile([P, NRC], f32)
                nc.scalar.activation(lse_r[:], rowsum[:], AF.Ln)
                lsum = sb.tile([P, 1], f32)
                nc.vector.reduce_sum(lsum[:], lse_r[:], axis=AX.X)
                pR = pcp.tile([P, CW], f32, tag="pc", name="pR")
                nc.tensor.matmul(pR[:1, :1], ones_kf[:], lsum[:])
                rpart = sb.tile([1, 1], f32)
                nc.vector.tensor_sub(rpart[:], pR[:1, :1], dsum[:])
                nc.sync.dma_start(ar_in[:1, b_total:b_total + 1], rpart[:])
                nc.sync.dma_start(ar_in[:1, b_total + 1:b_total + 2], dsum[:])
                nc.sync.dma_start(ar_in[:1, b_total + 2:ARW], zpad[:1, :62])

                # ---- AllReduce (second half + scalars) ----
                nc.gpsimd.collective_compute(
                    "AllReduce", Alu.add, replica_groups=rg,
                    ins=[ar_in[:1, ARH:ARW].opt()], outs=[ar_out_b.opt()])

                # ---- final ----
                # sum_j log(colsum_j) is order-independent, so the permuted
                # (pos-ordered) colsum layout needs no unscrambling.
                HB2 = b_total - ARH
                csb = sb.tile([P, b_total // P], f32)
                nc.sync.dma_start(
                    csb[:, :ARH // P],
                    ar_out_a.rearrange("o (a b) -> (o a) b", a=P))
                nc.sync.dma_start(
                    csb[:, ARH // P:],
                    ar_out_b[:1, :HB2].rearrange("o (a b) -> (o a) b", a=P))
                sc2 = sb.tile([1, 2], f32)
                nc.sync.dma_start(sc2[:], ar_out_b[:1, HB2:HB2 + 2])
                lse_c = sb.tile([P, b_total // P], f32)
                nc.scalar.activation(lse_c[:], csb[:], AF.Ln)
                csum_p = sb.tile([P, 1], f32)
                nc.vector.reduce_sum(csum_p[:], lse_c[:], axis=AX.X)
                pC = pcp.tile([P, CW], f32, tag="pc", name="pC")
                nc.tensor.matmul(pC[:1, :1], ones_kf[:], csum_p[:])
                t1 = sb.tile([1, 1], f32)
                nc.vector.tensor_add(t1[:], pC[:1, :1], sc2[:1, 0:1])
                t2 = sb.tile([1, 1], f32)
                nc.vector.tensor_sub(t2[:], t1[:], sc2[:1, 1:2])
                lsb = sb.tile([1, 1], f32)
                nc.vector.tensor_scalar_mul(lsb[:], t2[:], 1.0 / (2.0 * b_total))
                nc.sync.dma_start(loss, lsb[:])

    nc.compile()
    _NC_CACHE[key] = nc
    return nc


def shard_inputs(inputs, b_total=B, n_cores=N_CORES):
    BL = b_total // n_cores
    img = np.ascontiguousarray(np.asarray(inputs["img_feat"], dtype=np.float32))
    num = np.ascontiguousarray(np.asarray(inputs["num_feat"], dtype=np.float32))

    def mat(name):
        return np.ascontiguousarray(np.asarray(inputs[name], dtype=np.float32))

    def col(name):
        return np.ascontiguousarray(
            np.asarray(inputs[name], dtype=np.float32).reshape(P, 1))

    lt = np.asarray(inputs["log_temp"], dtype=np.float32).reshape(1, 1)
    shared = {
        "Wi1": mat("Wi1"), "Wi2": mat("Wi2"),
        "Wn1": mat("Wn1"), "Wn2": mat("Wn2"),
        "bi1": col("bi1"), "bi2": col("bi2"),
        "bn1": col("bn1"), "bn2": col("bn2"),
        "log_temp": np.ascontiguousarray(lt),
    }
    maps = []
    for c in range(n_cores):
        m = dict(shared)
        m["img_feat"] = np.ascontiguousarray(img[c * BL:(c + 1) * BL])
        m["num_feat"] = np.ascontiguousarray(num[c * BL:(c + 1) * BL])
        maps.append(m)
    return maps


def run(inputs, trace=False, **kw):
    """Run on hardware; returns (loss_scalar, BassKernelResults)."""
    from concourse.bass_utils import run_bass_kernel_spmd
    nc = build()
    res = run_bass_kernel_spmd(nc, shard_inputs(inputs),
                               core_ids=list(range(N_CORES)), trace=trace, **kw)
    val = np.asarray(res.results[0]["loss"], dtype=np.float32).reshape(())
    return val, res


def kernel(**inputs):
    val, _ = run(inputs)
    return val



# revision 5
# speedup vs baseline: 1.0891x; 1.0891x over previous
"""Trainium2 Bass/Tile kernel: symmetric contrastive loss (CLIP-style).

Distribution: data-parallel over B across 8 NeuronCores.  Each core MLPs +
l2-normalizes its 2048-row shard of both branches, AllGathers the normalized
num-projections (bf16, 512KB/rank), computes its row-block of the 16384^2
logit matrix tile-by-tile (never materialized), and reduces:

  * rows  (i2n): ACT Exp with fused accum_out -> per-row sum(exp) locally
  * cols  (n2i): DVE bf16 adds fold the 16 row-chunks of each 2048-wide
    column supertile into an SBUF accumulator; one ones-matmul per supertile
    turns it into colsums; two AllReduce-adds ([8192] mid-loop hidden,
    [8192+scalars] tail) finish the cross-core reduction.

v2 vs v1:
  * 2048-wide Exp tiles (128 instead of 256): amortizes the fixed ~650ns
    per-ACTIVATE overhead (SBUF access bubble + accumulator readout).
  * column sums moved off the PE (was 50% of PE time) onto otherwise-idle
    DVE; PE now only runs the logits matmuls in the main loop.
  * normalization Ln/Exp batched per branch so the ACT table stays on the
    exp set through the main loop (v1 thrashed 18 table loads).
  * dummy 8-element AllGather issued at t~0 absorbs the ~49us one-time
    collective bootstrap concurrently with the input DMAs.
  * single full AllGather issued as soon as ntl is ready.

Logits are bounded (|cos|/temp <= 10) so logsumexp needs no max shift; the
l2 normalization is exp(-0.5*ln(|z|^2) - 0.5*log_temp) on ACT (Rsqrt on ACT
is banned for accuracy), with temperature folded in via the bias.
"""

import numpy as np

N_CORES = 8
B = 16384
D_IMG = 2048
D_NUM = 256
P = 128

_NC_CACHE = {}


def build(b_total=B, d_img=D_IMG, d_num=D_NUM, n_cores=N_CORES):
    """Build + compile the Bass module. Returns the compiled Bacc object."""
    key = (b_total, d_img, d_num, n_cores)
    if key in _NC_CACHE:
        return _NC_CACHE[key]

    import concourse.bacc as bacc
    import concourse.bass as bass
    import concourse.mybir as mybir
    import concourse.tile as tile

    dt = mybir.dt
    AF = mybir.ActivationFunctionType
    Alu = mybir.AluOpType
    AX = mybir.AxisListType
    f32 = dt.float32
    bf16 = dt.bfloat16

    BL = b_total // n_cores          # local rows per core
    assert BL % 512 == 0 and b_total % 2048 == 0
    NRT = BL // 512                  # 512-wide row tiles (MLP)
    NRC = BL // 128                  # 128-row chunks (main pass)
    KI = d_img // 128                # contraction tiles, img MLP1
    KN = d_num // 128
    CW = 2048                        # main-pass column supertile width
    NCT = b_total // CW              # number of supertiles
    NH = CW // 512
    ARW = b_total + 64               # total AllReduce payload width
    ARH = (NCT // 2) * CW            # first AllReduce chunk (cols)

    nc = bacc.Bacc("TRN2", target_bir_lowering=False, debug=False,
                   num_devices=n_cores)

    img = nc.dram_tensor("img_feat", [BL, d_img], f32, kind="ExternalInput").ap()
    num = nc.dram_tensor("num_feat", [BL, d_num], f32, kind="ExternalInput").ap()
    Wi1 = nc.dram_tensor("Wi1", [d_img, P], f32, kind="ExternalInput").ap()
    bi1 = nc.dram_tensor("bi1", [P, 1], f32, kind="ExternalInput").ap()
    Wi2 = nc.dram_tensor("Wi2", [P, P], f32, kind="ExternalInput").ap()
    bi2 = nc.dram_tensor("bi2", [P, 1], f32, kind="ExternalInput").ap()
    Wn1 = nc.dram_tensor("Wn1", [d_num, P], f32, kind="ExternalInput").ap()
    bn1 = nc.dram_tensor("bn1", [P, 1], f32, kind="ExternalInput").ap()
    Wn2 = nc.dram_tensor("Wn2", [P, P], f32, kind="ExternalInput").ap()
    bn2 = nc.dram_tensor("bn2", [P, 1], f32, kind="ExternalInput").ap()
    ltm = nc.dram_tensor("log_temp", [1, 1], f32, kind="ExternalInput").ap()
    loss = nc.dram_tensor("loss", [1, 1], f32, kind="ExternalOutput").ap()

    rg = [list(range(n_cores))]

    with tile.TileContext(nc) as tc:
        with (
            tc.tile_pool(name="sb", bufs=1) as sb,
            tc.tile_pool(name="stream", bufs=3) as st,
            tc.tile_pool(name="vstage", bufs=2) as vs,
            tc.tile_pool(name="xtp", bufs=2) as xtp,
            tc.tile_pool(name="xsp", bufs=2) as xsp,
            tc.tile_pool(name="dram", bufs=1, space="DRAM") as dram,
        ):
            # ---------------- DRAM scratch ----------------
            dum_in = dram.tile([1, 8], f32)
            dum_out = dram.tile([n_cores, 8], f32, addr_space="Shared")
            ag_in = dram.tile([P, BL], bf16)
            ag_out = dram.tile([n_cores * P, BL], bf16, addr_space="Shared")
            ar_in = dram.tile([1, ARW], f32)
            ar_out_a = dram.tile([1, ARH], f32, addr_space="Shared")
            ar_out_b = dram.tile([1, ARW - ARH], f32, addr_space="Shared")

            # ---------------- constants ----------------
            zpad = sb.tile([1, 64], f32)
            nc.vector.memset(zpad[:], 0.0)
            # dummy collective first: absorbs the one-time comm bootstrap
            # (~49us) while the input DMAs and MLPs run.
            nc.sync.dma_start(dum_in[:], zpad[:1, 0:8])
            nc.gpsimd.collective_compute(
                "AllGather", Alu.bypass, replica_groups=rg,
                ins=[dum_in.opt()], outs=[dum_out.opt()])

            ones_kb = sb.tile([P, 1], bf16)
            nc.vector.memset(ones_kb[:], 1.0)
            ones_kf = sb.tile([P, 1], f32)
            nc.vector.memset(ones_kf[:], 1.0)
            ones_1f = sb.tile([1, P], f32)
            nc.vector.memset(ones_1f[:], 1.0)
            # identity (bf16) for PE-mode transposes: (free_idx - part_idx)==0
            idn_i = sb.tile([P, P], dt.int32)
            nc.gpsimd.iota(idn_i[:], pattern=[[1, P]], base=0,
                           channel_multiplier=-1)
            idn = sb.tile([P, P], bf16)
            nc.vector.tensor_scalar(idn[:], idn_i[:], 0, None,
                                    op0=Alu.is_equal)
            idn_f = sb.tile([P, P], f32)
            nc.vector.tensor_scalar(idn_f[:], idn_i[:], 0, None,
                                    op0=Alu.is_equal)

            # num input first -- it gates the whole AllGather chain.
            xs_n = sb.tile([P, NRC, d_num], f32)
            nc.sync.dma_start(xs_n[:], num.rearrange("(g p) e -> p g e", p=P))

            # num-branch weights: HWDGE (sync) as fp32 + DVE cast so they
            # never queue behind the big img SWDGE cast-DMAs.
            wn1_f = sb.tile([P, KN * P], f32)
            nc.sync.dma_start(wn1_f.rearrange("p (k m) -> p k m", k=KN),
                              Wn1.rearrange("(k p) m -> p k m", p=P))
            wn1_sb = sb.tile([P, KN * P], bf16)
            nc.vector.tensor_copy(wn1_sb[:], wn1_f[:])
            wn2_f = sb.tile([P, P], f32)
            nc.sync.dma_start(wn2_f[:], Wn2)
            wn2_sb = sb.tile([P, P], bf16)
            nc.vector.tensor_copy(wn2_sb[:], wn2_f[:])
            bn1_sb = sb.tile([P, 1], f32)
            nc.sync.dma_start(bn1_sb[:], bn1)
            bn2_sb = sb.tile([P, 1], f32)
            nc.sync.dma_start(bn2_sb[:], bn2)
            lt_sb = sb.tile([1, 1], f32)
            nc.sync.dma_start(lt_sb[:], ltm)
            nhlt = sb.tile([1, 1], f32)        # -0.5 * log_temp
            nc.vector.tensor_scalar_mul(nhlt[:], lt_sb[:], -0.5)

            # img input + weights: SWDGE cast-DMAs on the gpsimd queue,
            # issued before the AllGather trigger so they start at t~0.
            # Only NRT-1 chunks fit in the pool up front; the last chunk's
            # dma_start would stall the gpsimd queue (and hence the
            # AllGather trigger) waiting for a free buffer, so it is
            # issued after the AllGather instead.
            NUP = min(NRT, 2)
            xs_i = []
            for rb in range(NUP):
                xs = xsp.tile([P, 4, d_img], bf16, tag="xsi", name="xsi",
                              bufs=2)
                nc.gpsimd.dma_start(
                    xs[:], img[rb * 512:(rb + 1) * 512, :]
                    .rearrange("(q p) e -> p q e", p=P))
                xs_i.append(xs)
            wi1_sb = sb.tile([P, KI * P], bf16)
            nc.gpsimd.dma_start(wi1_sb.rearrange("p (k m) -> p k m", k=KI),
                                Wi1.rearrange("(k p) m -> p k m", p=P))
            wi2_sb = sb.tile([P, P], bf16)
            nc.gpsimd.dma_start(wi2_sb[:], Wi2)
            bi1_sb = sb.tile([P, 1], f32)
            nc.sync.dma_start(bi1_sb[:], bi1)
            bi2_sb = sb.tile([P, 1], f32)
            nc.sync.dma_start(bi2_sb[:], bi2)

            # ---------------- persistent SBUF ----------------
            xnT = sb.tile([P, KN * BL], bf16)   # num input, transposed
            h1n = sb.tile([P, BL], bf16)
            h1i = sb.tile([P, BL], bf16)
            zn = sb.tile([P, BL], bf16)
            zi = sb.tile([P, BL], bf16)
            ntl = sb.tile([P, BL], bf16)        # normalized num proj (local)
            itl = sb.tile([P, BL], bf16)        # normalized img proj (local)
            npf = sb.tile([P, b_total], bf16)   # gathered num proj (all cores)
            vsq = sb.tile([1, BL], f32)         # per-row |z|^2 staging
            inv = sb.tile([1, BL], f32)         # per-row 1/(|z| sqrt(temp))
            rowacc = sb.tile([P, NRC * NCT], f32)
            dsum = sb.tile([1, 1], f32)         # running sum of diag
            nc.vector.memset(dsum[:], 0.0)

            def mlp2_norm(pp, h1, w2, b2, z, outp):
                """z = w2.T@h1 + b2 (transposed layout); outp = z * inv, with
                inv[i] = exp(-0.5*ln(|z_i|^2) - 0.5*log_temp).  The Ln/Exp
                pair is batched over the whole branch so the ACT table set
                switches only twice per branch instead of twice per row
                tile."""
                for rt in range(NRT):
                    sl = slice(rt * 512, (rt + 1) * 512)
                    pz = pp.tile([P, 512], f32, tag="zb", name="pz")
                    nc.tensor.matmul(pz[:], w2[:], h1[:, sl])
                    nc.scalar.activation(z[:, sl], pz[:], AF.Identity, bias=b2[:])
                    sq = st.tile([P, 512], bf16, tag="sq", name="sq")
                    nc.scalar.activation(sq[:], pz[:], AF.Square, bias=b2[:])
                    pv = pp.tile([P, 512], f32, tag="v", name="pv")
                    nc.tensor.matmul(pv[:1, :], ones_kb[:], sq[:])
                    nc.vector.tensor_copy(vsq[:1, sl], pv[:1, :])
                nc.scalar.activation(inv[:], vsq[:], AF.Ln)
                nc.scalar.activation(inv[:], inv[:], AF.Exp,
                                     bias=nhlt[:], scale=-0.5)
                for rt in range(NRT):
                    sl = slice(rt * 512, (rt + 1) * 512)
                    pb = pp.tile([P, 512], f32, tag="zb", name="pb")
                    nc.tensor.matmul(pb[:], ones_1f[:], inv[:1, sl])
                    nc.vector.tensor_mul(outp[:, sl], z[:, sl], pb[:])

            # ---------------- num branch + AllGather ----------------
            with tc.tile_pool(name="ppn", bufs=2, space="PSUM") as pp:
                # f32 PE transposes + cast copy from the early xs_n load
                for dk in range(KN):
                    for gb in range(NRC // 4):
                        pt = pp.tile([P, 512], f32, tag="pt", name="ptn")
                        for q in range(4):
                            nc.tensor.transpose(
                                pt[:, q * P:(q + 1) * P],
                                xs_n[:, gb * 4 + q, dk * P:(dk + 1) * P],
                                idn_f[:])
                        nc.vector.tensor_copy(
                            xnT[:, dk * BL + gb * 512: dk * BL + gb * 512 + 512],
                            pt[:])
                for rt in range(NRT):
                    sl = slice(rt * 512, (rt + 1) * 512)
                    ph = pp.tile([P, 512], f32, tag="h", name="ph")
                    for k in range(KN):
                        nc.tensor.matmul(
                            ph[:], wn1_sb[:, k * P:(k + 1) * P],
                            xnT[:, k * BL + rt * 512: k * BL + rt * 512 + 512],
                            start=(k == 0), stop=(k == KN - 1))
                    nc.scalar.activation(h1n[:, sl], ph[:], AF.Relu, bias=bn1_sb[:])
                mlp2_norm(pp, h1n, wn2_sb, bn2_sb, zn, ntl)

            nc.sync.dma_start(ag_in[:], ntl[:])
            nc.gpsimd.collective_compute(
                "AllGather", Alu.bypass, replica_groups=rg,
                ins=[ag_in.opt()], outs=[ag_out.opt()])
            # remaining img chunks (see NUP comment above)
            for rb in range(NUP, NRT):
                xs = xsp.tile([P, 4, d_img], bf16, tag="xsi", name="xsi",
                              bufs=2)
                nc.gpsimd.dma_start(
                    xs[:], img[rb * 512:(rb + 1) * 512, :]
                    .rearrange("(q p) e -> p q e", p=P))
                xs_i.append(xs)
            # unpack per core block so low column blocks land first
            for c in range(n_cores):
                nc.sync.dma_start(npf[:, c * BL:(c + 1) * BL],
                                  ag_out[c * P:(c + 1) * P, :])

            # ---------------- img branch ----------------
            with tc.tile_pool(name="ppi", bufs=2, space="PSUM") as pp:
                for rb in range(NRT):
                    rsl = slice(rb * 512, (rb + 1) * 512)
                    xs = xs_i[rb]
                    xtb = xtp.tile([P, KI * 512], bf16, tag="xt", name="xtb")
                    for dk in range(KI):
                        pt = pp.tile([P, 512], bf16, tag="pt", name="pt")
                        for q in range(4):
                            nc.tensor.transpose(
                                pt[:, q * P:(q + 1) * P],
                                xs[:, q, dk * P:(dk + 1) * P], idn[:])
                        nc.vector.tensor_copy(
                            xtb[:, dk * 512:(dk + 1) * 512], pt[:])
                    ph = pp.tile([P, 512], f32, tag="h", name="phi")
                    for k in range(KI):
                        nc.tensor.matmul(
                            ph[:], wi1_sb[:, k * P:(k + 1) * P],
                            xtb[:, k * 512:(k + 1) * 512],
                            start=(k == 0), stop=(k == KI - 1))
                    nc.scalar.activation(h1i[:, rsl], ph[:], AF.Relu,
                                         bias=bi1_sb[:])
                mlp2_norm(pp, h1i, wi2_sb, bi2_sb, zi, itl)
                # diagonal: l_ii = sum_p itl[p,i] * ntl[p,i]; accumulate sum
                for rt in range(NRT):
                    sl = slice(rt * 512, (rt + 1) * 512)
                    prod = st.tile([P, 512], bf16, tag="sq", name="prod")
                    nc.vector.tensor_mul(prod[:], itl[:, sl], ntl[:, sl])
                    pd = pp.tile([P, 512], f32, tag="v", name="pd")
                    nc.tensor.matmul(pd[:1, :], ones_kb[:], prod[:])
                    dred = vs.tile([1, 1], f32, tag="dred", name="dred")
                    nc.vector.reduce_sum(dred[:], pd[:1, :], axis=AX.X)
                    nc.vector.tensor_add(dsum[:], dsum[:], dred[:])

            # ---------------- main pass ----------------
            # Per supertile ct (2048 cols): 16 row-chunk tiles. PE computes
            # logits into PSUM, ACT exponentiates (rowsums via accum_out),
            # DVE folds e-tiles into a bf16 column accumulator. The colsum
            # finalize (4 ones-matmuls) is deferred past the next ct's first
            # tile to keep it off the PE queue's critical path.
            with tc.tile_pool(name="pl", bufs=2, space="PSUM") as plp:
                pending = None          # (acc, ct) awaiting colsum finalize

                def finalize_colsum(acc, ct):
                    pcs = plp.tile([P, CW], f32, tag="pl", name="pcs")
                    for h in range(NH):
                        nc.tensor.matmul(
                            pcs[:1, h * 512:(h + 1) * 512],
                            ones_kb[:], acc[:, h * 512:(h + 1) * 512])
                    cst = vs.tile([1, CW], f32, tag="cst", name="cst", bufs=1)
                    nc.vector.tensor_copy(cst[:], pcs[:1, :])
                    nc.sync.dma_start(ar_in[:1, ct * CW:(ct + 1) * CW], cst[:])
                    if ct == NCT // 2 - 1:
                        # first half of colsums complete -> overlap AllReduce
                        nc.gpsimd.collective_compute(
                            "AllReduce", Alu.add, replica_groups=rg,
                            ins=[ar_in[:1, 0:ARH].opt()], outs=[ar_out_a.opt()])

                for ct in range(NCT):
                    acc = vs.tile([P, CW], bf16, tag="acc", name="acc")
                    for rc in range(NRC):
                        plog = plp.tile([P, CW], f32, tag="pl", name="plog")
                        for h in range(NH):
                            nc.tensor.matmul(
                                plog[:, h * 512:(h + 1) * 512],
                                itl[:, rc * P:(rc + 1) * P],
                                npf[:, ct * CW + h * 512: ct * CW + (h + 1) * 512])
                        e = st.tile([P, CW], bf16, tag="e", name="e", bufs=4)
                        slot = rc * NCT + ct
                        nc.scalar.activation(e[:], plog[:], AF.Exp,
                                             accum_out=rowacc[:, slot:slot + 1])
                        if rc == 0:
                            nc.vector.tensor_copy(acc[:], e[:])
                        else:
                            nc.vector.tensor_add(acc[:], acc[:], e[:])
                        if rc == 1 and pending is not None:
                            finalize_colsum(*pending)
                            pending = None
                    pending = (acc, ct)
                finalize_colsum(*pending)

                # ---- row direction partials ----
                rowsum = sb.tile([P, NRC], f32)
                nc.vector.reduce_sum(
                    rowsum[:],
                    rowacc.rearrange("p (rc ct) -> p rc ct", ct=NCT), axis=AX.X)
                lse_r = sb.tile([P, NRC], f32)
                nc.scalar.activation(lse_r[:], rowsum[:], AF.Ln)
                lsum = sb.tile([P, 1], f32)
                nc.vector.reduce_sum(lsum[:], lse_r[:], axis=AX.X)
                pR = plp.tile([P, CW], f32, tag="pl", name="pR")
                nc.tensor.matmul(pR[:1, :1], ones_kf[:], lsum[:])
                rpart = sb.tile([1, 1], f32)
                nc.vector.tensor_sub(rpart[:], pR[:1, :1], dsum[:])
                nc.sync.dma_start(ar_in[:1, b_total:b_total + 1], rpart[:])
                nc.sync.dma_start(ar_in[:1, b_total + 1:b_total + 2], dsum[:])
                nc.sync.dma_start(ar_in[:1, b_total + 2:ARW], zpad[:1, :62])

                # ---- AllReduce (second half + scalars) ----
                nc.gpsimd.collective_compute(
                    "AllReduce", Alu.add, replica_groups=rg,
                    ins=[ar_in[:1, ARH:ARW].opt()], outs=[ar_out_b.opt()])

                # ---- final ----
                # sum_j log(colsum_j) is order-independent, so any colsum
                # layout works.
                HB2 = b_total - ARH
                csb = sb.tile([P, b_total // P], f32)
                nc.sync.dma_start(
                    csb[:, :ARH // P],
                    ar_out_a.rearrange("o (a b) -> (o a) b", a=P))
                nc.sync.dma_start(
                    csb[:, ARH // P:],
                    ar_out_b[:1, :HB2].rearrange("o (a b) -> (o a) b", a=P))
                sc2 = sb.tile([1, 2], f32)
                nc.sync.dma_start(sc2[:], ar_out_b[:1, HB2:HB2 + 2])
                lse_c = sb.tile([P, b_total // P], f32)
                nc.scalar.activation(lse_c[:], csb[:], AF.Ln)
                csum_p = sb.tile([P, 1], f32)
                nc.vector.reduce_sum(csum_p[:], lse_c[:], axis=AX.X)
                pC = plp.tile([P, CW], f32, tag="pl", name="pC")
                nc.tensor.matmul(pC[:1, :1], ones_kf[:], csum_p[:])
                t1 = sb.tile([1, 1], f32)
                nc.vector.tensor_add(t1[:], pC[:1, :1], sc2[:1, 0:1])
                t2 = sb.tile([1, 1], f32)
                nc.vector.tensor_sub(t2[:], t1[:], sc2[:1, 1:2])
                lsb = sb.tile([1, 1], f32)
                nc.vector.tensor_scalar_mul(lsb[:], t2[:], 1.0 / (2.0 * b_total))
                nc.sync.dma_start(loss, lsb[:])

    nc.compile()
    _NC_CACHE[key] = nc
    return nc


def shard_inputs(inputs, b_total=B, n_cores=N_CORES):
    BL = b_total // n_cores
    img = np.ascontiguousarray(np.asarray(inputs["img_feat"], dtype=np.float32))
    num = np.ascontiguousarray(np.asarray(inputs["num_feat"], dtype=np.float32))

    def mat(name):
        return np.ascontiguousarray(np.asarray(inputs[name], dtype=np.float32))

    def col(name):
        return np.ascontiguousarray(
            np.asarray(inputs[name], dtype=np.float32).reshape(P, 1))

    lt = np.asarray(inputs["log_temp"], dtype=np.float32).reshape(1, 1)
    shared = {
        "Wi1": mat("Wi1"), "Wi2": mat("Wi2"),
        "Wn1": mat("Wn1"), "Wn2": mat("Wn2"),
        "bi1": col("bi1"), "bi2": col("bi2"),
        "bn1": col("bn1"), "bn2": col("bn2"),
        "log_temp": np.ascontiguousarray(lt),
    }
    maps = []
    for c in range(n_cores):
        m = dict(shared)
        m["img_feat"] = np.ascontiguousarray(img[c * BL:(c + 1) * BL])
        m["num_feat"] = np.ascontiguousarray(num[c * BL:(c + 1) * BL])
        maps.append(m)
    return maps


def run(inputs, trace=False, **kw):
    """Run on hardware; returns (loss_scalar, BassKernelResults)."""
    from concourse.bass_utils import run_bass_kernel_spmd
    nc = build()
    res = run_bass_kernel_spmd(nc, shard_inputs(inputs),
                               core_ids=list(range(N_CORES)), trace=trace, **kw)
    val = np.asarray(res.results[0]["loss"], dtype=np.float32).reshape(())
    return val, res


def kernel(**inputs):
    val, _ = run(inputs)
    return val


# revision 7
# speedup vs baseline: 1.3412x; 1.2315x over previous
"""Trainium2 Bass/Tile kernel: symmetric contrastive loss (CLIP-style).

Distribution: data-parallel over B across 8 NeuronCores.  Each core MLPs +
l2-normalizes its 2048-row shard of both branches, AllGathers the normalized
num-projections (bf16, 512KB/rank), computes its row-block of the 16384^2
logit matrix tile-by-tile (never materialized), and reduces:

  * rows  (i2n): ACT Exp with fused accum_out -> per-row sum(exp) locally
  * cols  (n2i): DVE bf16 adds fold the 16 row-chunks of each 2048-wide
    column supertile into an SBUF accumulator; one ones-matmul per supertile
    turns it into colsums; two AllReduce-adds ([8192] mid-loop hidden,
    [8192+scalars] tail) finish the cross-core reduction.

v3: inputs are staged host-side TRANSPOSED (feature-major), so the MLPs
read them directly as matmul operands -- no PE transposes, no PSUM bounce,
no transpose copies (v2 spent ~90us of PE prologue on 288 transposes).
The img MLP1 runs contraction-outer so each arriving k-chunk is consumed
immediately; img chunk DMAs alternate between the SWDGE (cast) and HWDGE
(f32 + DVE cast) queues to use both DMA paths in parallel.

Other structure (from v2): 2048-wide Exp tiles; column sums on DVE;
normalization Ln/Exp batched per branch (ACT stays on the exp table set
through the main loop); dummy 8-element AllGather at t~0 absorbs the
one-time collective bootstrap; colsums DMA'd straight from PSUM.

Logits are bounded (|cos|/temp <= 10) so logsumexp needs no max shift; the
l2 normalization is exp(-0.5*ln(|z|^2) - 0.5*log_temp) on ACT (Rsqrt on ACT
is banned for accuracy), with temperature folded in via the bias.
"""

import numpy as np

N_CORES = 8
B = 16384
D_IMG = 2048
D_NUM = 256
P = 128

_NC_CACHE = {}


def build(b_total=B, d_img=D_IMG, d_num=D_NUM, n_cores=N_CORES):
    """Build + compile the Bass module. Returns the compiled Bacc object."""
    key = (b_total, d_img, d_num, n_cores)
    if key in _NC_CACHE:
        return _NC_CACHE[key]

    import concourse.bacc as bacc
    import concourse.bass as bass
    import concourse.mybir as mybir
    import concourse.tile as tile

    dt = mybir.dt
    AF = mybir.ActivationFunctionType
    Alu = mybir.AluOpType
    AX = mybir.AxisListType
    f32 = dt.float32
    bf16 = dt.bfloat16

    BL = b_total // n_cores          # local rows per core
    assert BL % 512 == 0 and b_total % 2048 == 0
    NRT = BL // 512                  # 512-wide row tiles (MLP)
    NRC = BL // 128                  # 128-row chunks (main pass)
    KI = d_img // 128                # contraction tiles, img MLP1
    KN = d_num // 128
    CW = 2048                        # main-pass column supertile width
    NCT = b_total // CW              # number of supertiles
    NH = CW // 512
    ARW = b_total + 64               # total AllReduce payload width
    ARH = (NCT // 2) * CW            # first AllReduce chunk (cols)

    nc = bacc.Bacc("TRN2", target_bir_lowering=False, debug=False,
                   num_devices=n_cores)

    imgT = nc.dram_tensor("imgT", [d_img, BL], f32, kind="ExternalInput").ap()
    numT = nc.dram_tensor("numT", [d_num, BL], f32, kind="ExternalInput").ap()
    Wi1 = nc.dram_tensor("Wi1", [d_img, P], f32, kind="ExternalInput").ap()
    bi1 = nc.dram_tensor("bi1", [P, 1], f32, kind="ExternalInput").ap()
    Wi2 = nc.dram_tensor("Wi2", [P, P], f32, kind="ExternalInput").ap()
    bi2 = nc.dram_tensor("bi2", [P, 1], f32, kind="ExternalInput").ap()
    Wn1 = nc.dram_tensor("Wn1", [d_num, P], f32, kind="ExternalInput").ap()
    bn1 = nc.dram_tensor("bn1", [P, 1], f32, kind="ExternalInput").ap()
    Wn2 = nc.dram_tensor("Wn2", [P, P], f32, kind="ExternalInput").ap()
    bn2 = nc.dram_tensor("bn2", [P, 1], f32, kind="ExternalInput").ap()
    ltm = nc.dram_tensor("log_temp", [1, 1], f32, kind="ExternalInput").ap()
    loss = nc.dram_tensor("loss", [1, 1], f32, kind="ExternalOutput").ap()

    rg = [list(range(n_cores))]

    with tile.TileContext(nc) as tc:
        with (
            tc.tile_pool(name="sb", bufs=1) as sb,
            tc.tile_pool(name="stream", bufs=3) as st,
            tc.tile_pool(name="vstage", bufs=2) as vs,
            tc.tile_pool(name="xfp", bufs=2) as xfp,
            tc.tile_pool(name="dram", bufs=1, space="DRAM") as dram,
        ):
            # ---------------- DRAM scratch ----------------
            dum_in = dram.tile([1, 8], f32)
            dum_out = dram.tile([n_cores, 8], f32, addr_space="Shared")
            ag_in = dram.tile([P, BL], bf16)
            ag_out = dram.tile([n_cores * P, BL], bf16, addr_space="Shared")
            ar_in = dram.tile([1, ARW], f32)
            ar_out_a = dram.tile([1, ARH], f32, addr_space="Shared")
            ar_out_b = dram.tile([1, ARW - ARH], f32, addr_space="Shared")

            # ---------------- bootstrap ----------------
            zpad = sb.tile([1, 64], f32)
            nc.vector.memset(zpad[:], 0.0)
            # dummy collective first: absorbs the one-time comm bootstrap
            # (~40us) while the input DMAs and MLPs run.
            nc.sync.dma_start(dum_in[:], zpad[:1, 0:8])
            nc.gpsimd.collective_compute(
                "AllGather", Alu.bypass, replica_groups=rg,
                ins=[dum_in.opt()], outs=[dum_out.opt()])

            # num input: SWDGE cast-DMA, first on the gpsimd queue -- it
            # gates the whole AllGather chain.
            xnb = sb.tile([P, KN, BL], bf16)
            nc.gpsimd.dma_start(xnb[:], numT.rearrange("(k p) r -> p k r", p=P))
            # img weights next (needed when img MLP1 starts)
            wi1_sb = sb.tile([P, KI * P], bf16)
            nc.gpsimd.dma_start(wi1_sb.rearrange("p (k m) -> p k m", k=KI),
                                Wi1.rearrange("(k p) m -> p k m", p=P))
            wi2_sb = sb.tile([P, P], bf16)
            nc.gpsimd.dma_start(wi2_sb[:], Wi2)
            # img input, even k-chunks: SWDGE cast-DMAs
            xib = sb.tile([P, KI, BL], bf16)
            for k in range(0, KI, 2):
                nc.gpsimd.dma_start(xib[:, k, :], imgT[k * P:(k + 1) * P, :])

            # ---------------- constants / small loads (sync+DVE) --------
            ones_kb = sb.tile([P, 1], bf16)
            nc.vector.memset(ones_kb[:], 1.0)
            ones_kf = sb.tile([P, 1], f32)
            nc.vector.memset(ones_kf[:], 1.0)
            ones_1b = sb.tile([1, P], bf16)
            nc.vector.memset(ones_1b[:], 1.0)
            nc.sync.dma_start(ar_in[:1, b_total + 2:ARW], zpad[:1, :62])

            wn1_f = sb.tile([P, KN * P], f32)
            nc.sync.dma_start(wn1_f.rearrange("p (k m) -> p k m", k=KN),
                              Wn1.rearrange("(k p) m -> p k m", p=P))
            wn1_sb = sb.tile([P, KN * P], bf16)
            nc.vector.tensor_copy(wn1_sb[:], wn1_f[:])
            wn2_f = sb.tile([P, P], f32)
            nc.sync.dma_start(wn2_f[:], Wn2)
            wn2_sb = sb.tile([P, P], bf16)
            nc.vector.tensor_copy(wn2_sb[:], wn2_f[:])
            bn1_sb = sb.tile([P, 1], f32)
            nc.sync.dma_start(bn1_sb[:], bn1)
            bn2_sb = sb.tile([P, 1], f32)
            nc.sync.dma_start(bn2_sb[:], bn2)
            lt_sb = sb.tile([1, 1], f32)
            nc.sync.dma_start(lt_sb[:], ltm)
            nhlt = sb.tile([1, 1], f32)        # -0.5 * log_temp
            nc.vector.tensor_scalar_mul(nhlt[:], lt_sb[:], -0.5)
            bi1_sb = sb.tile([P, 1], f32)
            nc.sync.dma_start(bi1_sb[:], bi1)
            bi2_sb = sb.tile([P, 1], f32)
            nc.sync.dma_start(bi2_sb[:], bi2)

            # img input, odd k-chunks: HWDGE f32 loads (cast on DVE later,
            # after the num branch's DVE work)
            xf_list = []
            for k in range(1, KI, 2):
                xf = xfp.tile([P, BL], f32, tag="xf", name="xf")
                nc.sync.dma_start(xf[:], imgT[k * P:(k + 1) * P, :])
                xf_list.append((k, xf))

            # ---------------- persistent SBUF ----------------
            h1n = sb.tile([P, BL], bf16)
            h1i = sb.tile([P, BL], bf16)
            zn = sb.tile([P, BL], bf16)
            zi = sb.tile([P, BL], bf16)
            ntl = sb.tile([P, BL], bf16)        # normalized num proj (local)
            itl = sb.tile([P, BL], bf16)        # normalized img proj (local)
            npf = sb.tile([P, b_total], bf16)   # gathered num proj (all cores)
            vsq = sb.tile([1, BL], f32)         # per-row |z|^2 staging
            inv_b = sb.tile([1, BL], bf16)      # per-row 1/(|z| sqrt(temp))
            rowacc = sb.tile([P, NRC * NCT], f32)
            dsum = sb.tile([1, 1], f32)         # running sum of diag
            nc.vector.memset(dsum[:], 0.0)

            def mlp2_norm(pp, h1, w2, b2, z, outp):
                """z = w2.T@h1 + b2 (transposed layout); outp = z * inv, with
                inv[i] = exp(-0.5*ln(|z_i|^2) - 0.5*log_temp).  The Ln/Exp
                pair is batched over the whole branch so the ACT table set
                switches only twice per branch instead of twice per row
                tile."""
                for rt in range(NRT):
                    sl = slice(rt * 512, (rt + 1) * 512)
                    pz = pp.tile([P, 512], f32, tag="zb", name="pz")
                    nc.tensor.matmul(pz[:], w2[:], h1[:, sl])
                    nc.scalar.activation(z[:, sl], pz[:], AF.Identity, bias=b2[:])
                    sq = st.tile([P, 512], bf16, tag="sq", name="sq")
                    nc.scalar.activation(sq[:], pz[:], AF.Square, bias=b2[:])
                    pv = pp.tile([P, 512], f32, tag="v", name="pv")
                    nc.tensor.matmul(pv[:1, :], ones_kb[:], sq[:])
                    nc.vector.tensor_copy(vsq[:1, sl], pv[:1, :])
                nc.scalar.activation(vsq[:], vsq[:], AF.Ln)
                nc.scalar.activation(inv_b[:], vsq[:], AF.Exp,
                                     bias=nhlt[:], scale=-0.5)
                for rt in range(NRT):
                    sl = slice(rt * 512, (rt + 1) * 512)
                    pb = pp.tile([P, 512], f32, tag="zb", name="pb")
                    nc.tensor.matmul(pb[:], ones_1b[:], inv_b[:1, sl])
                    nc.vector.tensor_mul(outp[:, sl], z[:, sl], pb[:])

            with tc.tile_pool(name="pp", bufs=2, space="PSUM") as pp:
                # ---------------- num branch + AllGather ----------------
                for rt in range(NRT):
                    sl = slice(rt * 512, (rt + 1) * 512)
                    ph = pp.tile([P, 512], f32, tag="h", name="ph", bufs=4)
                    for k in range(KN):
                        nc.tensor.matmul(ph[:], wn1_sb[:, k * P:(k + 1) * P],
                                         xnb[:, k, sl],
                                         start=(k == 0), stop=(k == KN - 1))
                    nc.scalar.activation(h1n[:, sl], ph[:], AF.Relu,
                                         bias=bn1_sb[:])
                mlp2_norm(pp, h1n, wn2_sb, bn2_sb, zn, ntl)

                # ag_in DMA rides the scalar queue (HWDGE): the scalar
                # engine reaches it right after the num branch finishes,
                # and the sync queue is busy streaming img f32 chunks.
                nc.scalar.dma_start(ag_in[:], ntl[:])
                nc.gpsimd.collective_compute(
                    "AllGather", Alu.bypass, replica_groups=rg,
                    ins=[ag_in.opt()], outs=[ag_out.opt()])
                # unpack per core block; ct0 lands first
                for c in range(n_cores):
                    nc.sync.dma_start(npf[:, c * BL:(c + 1) * BL],
                                      ag_out[c * P:(c + 1) * P, :])

                # DVE casts for the odd img chunks (after num DVE work)
                for k, xf in xf_list:
                    nc.vector.tensor_copy(xib[:, k, :], xf[:])

                # ---------------- img branch ----------------
                # contraction-outer MLP1: each k-chunk is consumed as it
                # arrives; the NRT row-tiles accumulate in parallel PSUM
                # banks across the whole k loop.
                ph_list = [pp.tile([P, 512], f32, tag="h", name=f"phi{rt}",
                                   bufs=4) for rt in range(NRT)]
                for k in range(KI):
                    for rt in range(NRT):
                        nc.tensor.matmul(
                            ph_list[rt][:], wi1_sb[:, k * P:(k + 1) * P],
                            xib[:, k, rt * 512:(rt + 1) * 512],
                            start=(k == 0), stop=(k == KI - 1))
                for rt in range(NRT):
                    nc.scalar.activation(h1i[:, rt * 512:(rt + 1) * 512],
                                         ph_list[rt][:], AF.Relu,
                                         bias=bi1_sb[:])
                mlp2_norm(pp, h1i, wi2_sb, bi2_sb, zi, itl)
                # diagonal: l_ii = sum_p itl[p,i] * ntl[p,i]; accumulate sum
                for rt in range(NRT):
                    sl = slice(rt * 512, (rt + 1) * 512)
                    prod = st.tile([P, 512], bf16, tag="sq", name="prod")
                    nc.vector.tensor_mul(prod[:], itl[:, sl], ntl[:, sl])
                    pd = pp.tile([P, 512], f32, tag="v", name="pd")
                    nc.tensor.matmul(pd[:1, :], ones_kb[:], prod[:])
                    dred = vs.tile([1, 1], f32, tag="dred", name="dred")
                    nc.vector.reduce_sum(dred[:], pd[:1, :], axis=AX.X)
                    nc.vector.tensor_add(dsum[:], dsum[:], dred[:])

            # ---------------- main pass ----------------
            # Per supertile ct (2048 cols): 16 row-chunk tiles. PE computes
            # logits into PSUM, ACT exponentiates (rowsums via accum_out),
            # DVE folds e-tiles into a bf16 column accumulator. The colsum
            # finalize (4 ones-matmuls) is deferred past the next ct's first
            # tile to keep it off the PE queue's critical path; colsums are
            # DMA'd to the AllReduce buffer straight from PSUM.
            with tc.tile_pool(name="pl", bufs=2, space="PSUM") as plp:
                pending = None          # (acc, ct) awaiting colsum finalize

                def finalize_colsum(acc, ct):
                    pcs = plp.tile([P, CW], f32, tag="pl", name="pcs")
                    for h in range(NH):
                        nc.tensor.matmul(
                            pcs[:1, h * 512:(h + 1) * 512],
                            ones_kb[:], acc[:, h * 512:(h + 1) * 512])
                    cst = vs.tile([1, CW], f32, tag="cst", name="cst", bufs=1)
                    nc.vector.tensor_copy(cst[:], pcs[:1, :])
                    nc.sync.dma_start(ar_in[:1, ct * CW:(ct + 1) * CW], cst[:])
                    if ct == NCT // 2 - 1:
                        # first half of colsums complete -> overlap AllReduce
                        nc.gpsimd.collective_compute(
                            "AllReduce", Alu.add, replica_groups=rg,
                            ins=[ar_in[:1, 0:ARH].opt()], outs=[ar_out_a.opt()])

                for ct in range(NCT):
                    acc = vs.tile([P, CW], bf16, tag="acc", name="acc")
                    for rc in range(NRC):
                        plog = plp.tile([P, CW], f32, tag="pl", name="plog")
                        for h in range(NH):
                            nc.tensor.matmul(
                                plog[:, h * 512:(h + 1) * 512],
                                itl[:, rc * P:(rc + 1) * P],
                                npf[:, ct * CW + h * 512: ct * CW + (h + 1) * 512])
                        e = st.tile([P, CW], bf16, tag="e", name="e", bufs=4)
                        slot = rc * NCT + ct
                        nc.scalar.activation(e[:], plog[:], AF.Exp,
                                             accum_out=rowacc[:, slot:slot + 1])
                        if rc == 0:
                            nc.vector.tensor_copy(acc[:], e[:])
                        else:
                            nc.vector.tensor_add(acc[:], acc[:], e[:])
                        if rc == 1 and pending is not None:
                            finalize_colsum(*pending)
                            pending = None
                    pending = (acc, ct)
                finalize_colsum(*pending)

                # ---- row direction partials ----
                rowsum = sb.tile([P, NRC], f32)
                nc.vector.reduce_sum(
                    rowsum[:],
                    rowacc.rearrange("p (rc ct) -> p rc ct", ct=NCT), axis=AX.X)
                lse_r = sb.tile([P, NRC], f32)
                nc.scalar.activation(lse_r[:], rowsum[:], AF.Ln)
                lsum = sb.tile([P, 1], f32)
                nc.vector.reduce_sum(lsum[:], lse_r[:], axis=AX.X)
                pR = plp.tile([P, CW], f32, tag="pl", name="pR")
                nc.tensor.matmul(pR[:1, :1], ones_kf[:], lsum[:])
                rpart = sb.tile([1, 1], f32)
                nc.vector.tensor_sub(rpart[:], pR[:1, :1], dsum[:])
                nc.sync.dma_start(ar_in[:1, b_total:b_total + 1], rpart[:])
                nc.sync.dma_start(ar_in[:1, b_total + 1:b_total + 2], dsum[:])

                # ---- AllReduce (second half + scalars) ----
                nc.gpsimd.collective_compute(
                    "AllReduce", Alu.add, replica_groups=rg,
                    ins=[ar_in[:1, ARH:ARW].opt()], outs=[ar_out_b.opt()])

                # ---- final ----
                # sum_j log(colsum_j) is order-independent, so any colsum
                # layout works.
                HB2 = b_total - ARH
                csb = sb.tile([P, b_total // P], f32)
                nc.sync.dma_start(
                    csb[:, :ARH // P],
                    ar_out_a.rearrange("o (a b) -> (o a) b", a=P))
                nc.sync.dma_start(
                    csb[:, ARH // P:],
                    ar_out_b[:1, :HB2].rearrange("o (a b) -> (o a) b", a=P))
                sc2 = sb.tile([1, 2], f32)
                nc.sync.dma_start(sc2[:], ar_out_b[:1, HB2:HB2 + 2])
                lse_c = sb.tile([P, b_total // P], f32)
                nc.scalar.activation(lse_c[:], csb[:], AF.Ln)
                csum_p = sb.tile([P, 1], f32)
                nc.vector.reduce_sum(csum_p[:], lse_c[:], axis=AX.X)
                pC = plp.tile([P, CW], f32, tag="pl", name="pC")
                nc.tensor.matmul(pC[:1, :1], ones_kf[:], csum_p[:])
                t1 = sb.tile([1, 1], f32)
                nc.vector.tensor_add(t1[:], pC[:1, :1], sc2[:1, 0:1])
                t2 = sb.tile([1, 1], f32)
                nc.vector.tensor_sub(t2[:], t1[:], sc2[:1, 1:2])
                lsb = sb.tile([1, 1], f32)
                nc.vector.tensor_scalar_mul(lsb[:], t2[:], 1.0 / (2.0 * b_total))
                nc.sync.dma_start(loss, lsb[:])

    nc.compile()
    _NC_CACHE[key] = nc
    return nc


def shard_inputs(inputs, b_total=B, n_cores=N_CORES):
    BL = b_total // n_cores
    img = np.asarray(inputs["img_feat"], dtype=np.float32)
    num = np.asarray(inputs["num_feat"], dtype=np.float32)

    def mat(name):
        return np.ascontiguousarray(np.asarray(inputs[name], dtype=np.float32))

    def col(name):
        return np.ascontiguousarray(
            np.asarray(inputs[name], dtype=np.float32).reshape(P, 1))

    lt = np.asarray(inputs["log_temp"], dtype=np.float32).reshape(1, 1)
    shared = {
        "Wi1": mat("Wi1"), "Wi2": mat("Wi2"),
        "Wn1": mat("Wn1"), "Wn2": mat("Wn2"),
        "bi1": col("bi1"), "bi2": col("bi2"),
        "bn1": col("bn1"), "bn2": col("bn2"),
        "log_temp": np.ascontiguousarray(lt),
    }
    maps = []
    for c in range(n_cores):
        m = dict(shared)
        m["imgT"] = np.ascontiguousarray(img[c * BL:(c + 1) * BL].T)
        m["numT"] = np.ascontiguousarray(num[c * BL:(c + 1) * BL].T)
        maps.append(m)
    return maps


def run(inputs, trace=False, **kw):
    """Run on hardware; returns (loss_scalar, BassKernelResults)."""
    from concourse.bass_utils import run_bass_kernel_spmd
    nc = build()
    res = run_bass_kernel_spmd(nc, shard_inputs(inputs),
                               core_ids=list(range(N_CORES)), trace=trace, **kw)
    val = np.asarray(res.results[0]["loss"], dtype=np.float32).reshape(())
    return val, res


def kernel(**inputs):
    val, _ = run(inputs)
    return val


# revision 8
# speedup vs baseline: 1.3865x; 1.0338x over previous
"""Trainium2 Bass/Tile kernel: symmetric contrastive loss (CLIP-style).

Distribution: data-parallel over B across 8 NeuronCores.  Each core MLPs +
l2-normalizes its 2048-row shard of both branches, AllGathers the normalized
num-projections (bf16, 512KB/rank), computes its row-block of the 16384^2
logit matrix tile-by-tile (never materialized), and reduces:

  * rows  (i2n): ACT Exp with fused accum_out -> per-row sum(exp) locally
  * cols  (n2i): DVE bf16 adds fold the 16 row-chunks of each 2048-wide
    column supertile into an SBUF accumulator; one ones-matmul per supertile
    turns it into colsums; two AllReduce-adds ([8192] mid-loop hidden,
    [8192+scalars] tail) finish the cross-core reduction.

v3: inputs are staged host-side TRANSPOSED (feature-major), so the MLPs
read them directly as matmul operands -- no PE transposes, no PSUM bounce,
no transpose copies (v2 spent ~90us of PE prologue on 288 transposes).
The img MLP1 runs contraction-outer so each arriving k-chunk is consumed
immediately; img chunk DMAs alternate between the SWDGE (cast) and HWDGE
(f32 + DVE cast) queues to use both DMA paths in parallel.

Other structure (from v2): 2048-wide Exp tiles; column sums on DVE;
normalization Ln/Exp batched per branch (ACT stays on the exp table set
through the main loop); dummy 8-element AllGather at t~0 absorbs the
one-time collective bootstrap; colsums DMA'd straight from PSUM.

Logits are bounded (|cos|/temp <= 10) so logsumexp needs no max shift; the
l2 normalization is exp(-0.5*ln(|z|^2) - 0.5*log_temp) on ACT (Rsqrt on ACT
is banned for accuracy), with temperature folded in via the bias.
"""

import numpy as np

N_CORES = 8
B = 16384
D_IMG = 2048
D_NUM = 256
P = 128

_NC_CACHE = {}


def build(b_total=B, d_img=D_IMG, d_num=D_NUM, n_cores=N_CORES):
    """Build + compile the Bass module. Returns the compiled Bacc object."""
    key = (b_total, d_img, d_num, n_cores)
    if key in _NC_CACHE:
        return _NC_CACHE[key]

    import concourse.bacc as bacc
    import concourse.bass as bass
    import concourse.mybir as mybir
    import concourse.tile as tile

    dt = mybir.dt
    AF = mybir.ActivationFunctionType
    Alu = mybir.AluOpType
    AX = mybir.AxisListType
    f32 = dt.float32
    bf16 = dt.bfloat16

    BL = b_total // n_cores          # local rows per core
    assert BL % 512 == 0 and b_total % 2048 == 0
    NRT = BL // 512                  # 512-wide row tiles (MLP)
    NRC = BL // 128                  # 128-row chunks (main pass)
    KI = d_img // 128                # contraction tiles, img MLP1
    KN = d_num // 128
    CW = 2048                        # main-pass column supertile width
    NCT = b_total // CW              # number of supertiles
    NH = CW // 512
    ARW = b_total + 64               # total AllReduce payload width
    ARH = (NCT // 2) * CW            # first AllReduce chunk (cols)

    nc = bacc.Bacc("TRN2", target_bir_lowering=False, debug=False,
                   num_devices=n_cores)

    imgT = nc.dram_tensor("imgT", [d_img, BL], bf16, kind="ExternalInput").ap()
    numT = nc.dram_tensor("numT", [d_num, BL], bf16, kind="ExternalInput").ap()
    Wi1 = nc.dram_tensor("Wi1", [d_img, P], bf16, kind="ExternalInput").ap()
    bi1 = nc.dram_tensor("bi1", [P, 1], f32, kind="ExternalInput").ap()
    Wi2 = nc.dram_tensor("Wi2", [P, P], bf16, kind="ExternalInput").ap()
    bi2 = nc.dram_tensor("bi2", [P, 1], f32, kind="ExternalInput").ap()
    Wn1 = nc.dram_tensor("Wn1", [d_num, P], bf16, kind="ExternalInput").ap()
    bn1 = nc.dram_tensor("bn1", [P, 1], f32, kind="ExternalInput").ap()
    Wn2 = nc.dram_tensor("Wn2", [P, P], bf16, kind="ExternalInput").ap()
    bn2 = nc.dram_tensor("bn2", [P, 1], f32, kind="ExternalInput").ap()
    ltm = nc.dram_tensor("log_temp", [1, 1], f32, kind="ExternalInput").ap()
    loss = nc.dram_tensor("loss", [1, 1], f32, kind="ExternalOutput").ap()

    rg = [list(range(n_cores))]

    with tile.TileContext(nc) as tc:
        with (
            tc.tile_pool(name="sb", bufs=1) as sb,
            tc.tile_pool(name="stream", bufs=3) as st,
            tc.tile_pool(name="vstage", bufs=2) as vs,
            tc.tile_pool(name="dram", bufs=1, space="DRAM") as dram,
        ):
            # ---------------- DRAM scratch ----------------
            dum_in = dram.tile([1, 8], f32)
            dum_out = dram.tile([n_cores, 8], f32, addr_space="Shared")
            ag_in = dram.tile([P, BL], bf16)
            ag_out = dram.tile([n_cores * P, BL], bf16, addr_space="Shared")
            ar_in = dram.tile([1, ARW], f32)
            ar_out_a = dram.tile([1, ARH], f32, addr_space="Shared")
            ar_out_b = dram.tile([1, ARW - ARH], f32, addr_space="Shared")

            # ---------------- bootstrap ----------------
            zpad = sb.tile([1, 64], f32)
            nc.vector.memset(zpad[:], 0.0)
            # dummy collective first: absorbs the one-time comm bootstrap
            # (~40us) while the input DMAs and MLPs run.
            nc.sync.dma_start(dum_in[:], zpad[:1, 0:8])
            nc.gpsimd.collective_compute(
                "AllGather", Alu.bypass, replica_groups=rg,
                ins=[dum_in.opt()], outs=[dum_out.opt()])

            # num input: SWDGE cast-DMA, first on the gpsimd queue -- it
            # gates the whole AllGather chain.
            xnb = sb.tile([P, KN, BL], bf16)
            nc.gpsimd.dma_start(xnb[:], numT.rearrange("(k p) r -> p k r", p=P))
            # img weights next (needed when img MLP1 starts)
            wi1_sb = sb.tile([P, KI * P], bf16)
            nc.gpsimd.dma_start(wi1_sb.rearrange("p (k m) -> p k m", k=KI),
                                Wi1.rearrange("(k p) m -> p k m", p=P))
            wi2_sb = sb.tile([P, P], bf16)
            nc.gpsimd.dma_start(wi2_sb[:], Wi2)
            # img input, even k-chunks: SWDGE cast-DMAs
            xib = sb.tile([P, KI, BL], bf16)
            for k in range(0, KI, 2):
                nc.gpsimd.dma_start(xib[:, k, :], imgT[k * P:(k + 1) * P, :])

            # ---------------- constants / small loads (sync+DVE) --------
            ones_kb = sb.tile([P, 1], bf16)
            nc.vector.memset(ones_kb[:], 1.0)
            ones_kf = sb.tile([P, 1], f32)
            nc.vector.memset(ones_kf[:], 1.0)
            ones_1b = sb.tile([1, P], bf16)
            nc.vector.memset(ones_1b[:], 1.0)
            nc.sync.dma_start(ar_in[:1, b_total + 2:ARW], zpad[:1, :62])

            wn1_sb = sb.tile([P, KN * P], bf16)
            nc.sync.dma_start(wn1_sb.rearrange("p (k m) -> p k m", k=KN),
                              Wn1.rearrange("(k p) m -> p k m", p=P))
            wn2_sb = sb.tile([P, P], bf16)
            nc.sync.dma_start(wn2_sb[:], Wn2)
            bn1_sb = sb.tile([P, 1], f32)
            nc.sync.dma_start(bn1_sb[:], bn1)
            bn2_sb = sb.tile([P, 1], f32)
            nc.sync.dma_start(bn2_sb[:], bn2)
            lt_sb = sb.tile([1, 1], f32)
            nc.sync.dma_start(lt_sb[:], ltm)
            nhlt = sb.tile([1, 1], f32)        # -0.5 * log_temp
            nc.vector.tensor_scalar_mul(nhlt[:], lt_sb[:], -0.5)
            bi1_sb = sb.tile([P, 1], f32)
            nc.sync.dma_start(bi1_sb[:], bi1)
            bi2_sb = sb.tile([P, 1], f32)
            nc.sync.dma_start(bi2_sb[:], bi2)

            # img input, odd k-chunks: HWDGE loads straight into xib
            for k in range(1, KI, 2):
                nc.sync.dma_start(xib[:, k, :], imgT[k * P:(k + 1) * P, :])

            # ---------------- persistent SBUF ----------------
            h1n = sb.tile([P, BL], bf16)
            h1i = sb.tile([P, BL], bf16)
            zn = sb.tile([P, BL], bf16)
            zi = sb.tile([P, BL], bf16)
            ntl = sb.tile([P, BL], bf16)        # normalized num proj (local)
            itl = sb.tile([P, BL], bf16)        # normalized img proj (local)
            npf = sb.tile([P, b_total], bf16)   # gathered num proj (all cores)
            vsq = sb.tile([1, BL], f32)         # per-row |z|^2 staging
            inv_b = sb.tile([1, BL], bf16)      # per-row 1/(|z| sqrt(temp))
            rowacc = sb.tile([P, NRC * NCT], f32)
            csb = sb.tile([P, b_total // P], f32)
            dsum = sb.tile([1, 1], f32)         # running sum of diag
            nc.vector.memset(dsum[:], 0.0)

            def mlp2_norm(pp, h1, w2, b2, z, outp):
                """z = w2.T@h1 + b2 (transposed layout); outp = z * inv, with
                inv[i] = exp(-0.5*ln(|z_i|^2) - 0.5*log_temp).  The Ln/Exp
                pair is batched over the whole branch so the ACT table set
                switches only twice per branch instead of twice per row
                tile."""
                for rt in range(NRT):
                    sl = slice(rt * 512, (rt + 1) * 512)
                    pz = pp.tile([P, 512], f32, tag="zb", name="pz")
                    nc.tensor.matmul(pz[:], w2[:], h1[:, sl])
                    nc.scalar.activation(z[:, sl], pz[:], AF.Identity, bias=b2[:])
                    sq = st.tile([P, 512], bf16, tag="sq", name="sq")
                    nc.scalar.activation(sq[:], pz[:], AF.Square, bias=b2[:])
                    pv = pp.tile([P, 512], f32, tag="v", name="pv")
                    nc.tensor.matmul(pv[:1, :], ones_kb[:], sq[:])
                    nc.vector.tensor_copy(vsq[:1, sl], pv[:1, :])
                nc.scalar.activation(vsq[:], vsq[:], AF.Ln)
                nc.scalar.activation(inv_b[:], vsq[:], AF.Exp,
                                     bias=nhlt[:], scale=-0.5)
                for rt in range(NRT):
                    sl = slice(rt * 512, (rt + 1) * 512)
                    pb = pp.tile([P, 512], f32, tag="zb", name="pb")
                    nc.tensor.matmul(pb[:], ones_1b[:], inv_b[:1, sl])
                    nc.vector.tensor_mul(outp[:, sl], z[:, sl], pb[:])

            with tc.tile_pool(name="pp", bufs=2, space="PSUM") as pp:
                # ---------------- num branch + AllGather ----------------
                for rt in range(NRT):
                    sl = slice(rt * 512, (rt + 1) * 512)
                    ph = pp.tile([P, 512], f32, tag="h", name="ph", bufs=4)
                    for k in range(KN):
                        nc.tensor.matmul(ph[:], wn1_sb[:, k * P:(k + 1) * P],
                                         xnb[:, k, sl],
                                         start=(k == 0), stop=(k == KN - 1))
                    nc.scalar.activation(h1n[:, sl], ph[:], AF.Relu,
                                         bias=bn1_sb[:])
                mlp2_norm(pp, h1n, wn2_sb, bn2_sb, zn, ntl)

                # ag_in DMA rides the scalar queue (HWDGE): the scalar
                # engine reaches it right after the num branch finishes,
                # and the sync queue is busy streaming img f32 chunks.
                nc.scalar.dma_start(ag_in[:], ntl[:])
                nc.gpsimd.collective_compute(
                    "AllGather", Alu.bypass, replica_groups=rg,
                    ins=[ag_in.opt()], outs=[ag_out.opt()])
                # unpack per core block; ct0 lands first
                for c in range(n_cores):
                    nc.sync.dma_start(npf[:, c * BL:(c + 1) * BL],
                                      ag_out[c * P:(c + 1) * P, :])

                # ---------------- img branch ----------------
                # contraction-outer MLP1: each k-chunk is consumed as it
                # arrives; the NRT row-tiles accumulate in parallel PSUM
                # banks across the whole k loop.
                ph_list = [pp.tile([P, 512], f32, tag="h", name=f"phi{rt}",
                                   bufs=4) for rt in range(NRT)]
                for k in range(KI):
                    for rt in range(NRT):
                        nc.tensor.matmul(
                            ph_list[rt][:], wi1_sb[:, k * P:(k + 1) * P],
                            xib[:, k, rt * 512:(rt + 1) * 512],
                            start=(k == 0), stop=(k == KI - 1))
                for rt in range(NRT):
                    nc.scalar.activation(h1i[:, rt * 512:(rt + 1) * 512],
                                         ph_list[rt][:], AF.Relu,
                                         bias=bi1_sb[:])
                mlp2_norm(pp, h1i, wi2_sb, bi2_sb, zi, itl)
                # diagonal: l_ii = sum_p itl[p,i] * ntl[p,i]; accumulate sum
                for rt in range(NRT):
                    sl = slice(rt * 512, (rt + 1) * 512)
                    prod = st.tile([P, 512], bf16, tag="sq", name="prod")
                    nc.vector.tensor_mul(prod[:], itl[:, sl], ntl[:, sl])
                    pd = pp.tile([P, 512], f32, tag="v", name="pd")
                    nc.tensor.matmul(pd[:1, :], ones_kb[:], prod[:])
                    dred = vs.tile([1, 1], f32, tag="dred", name="dred")
                    nc.vector.reduce_sum(dred[:], pd[:1, :], axis=AX.X)
                    nc.vector.tensor_add(dsum[:], dsum[:], dred[:])

            # ---------------- main pass ----------------
            # Per supertile ct (2048 cols): 16 row-chunk tiles. PE computes
            # logits into PSUM, ACT exponentiates (rowsums via accum_out),
            # DVE folds e-tiles into a bf16 column accumulator. The colsum
            # finalize (4 ones-matmuls) is deferred past the next ct's first
            # tile to keep it off the PE queue's critical path; colsums are
            # DMA'd to the AllReduce buffer straight from PSUM.
            with tc.tile_pool(name="pl", bufs=2, space="PSUM") as plp:
                pending = None          # (acc, ct) awaiting colsum finalize

                def finalize_colsum(acc, ct):
                    pcs = plp.tile([P, CW], f32, tag="pl", name="pcs")
                    for h in range(NH):
                        nc.tensor.matmul(
                            pcs[:1, h * 512:(h + 1) * 512],
                            ones_kb[:], acc[:, h * 512:(h + 1) * 512])
                    cst = vs.tile([1, CW], f32, tag="cst", name="cst", bufs=1)
                    nc.vector.tensor_copy(cst[:], pcs[:1, :])
                    nc.sync.dma_start(ar_in[:1, ct * CW:(ct + 1) * CW], cst[:])
                    if ct == NCT // 2 - 1:
                        # first half of colsums complete -> overlap AllReduce
                        nc.gpsimd.collective_compute(
                            "AllReduce", Alu.add, replica_groups=rg,
                            ins=[ar_in[:1, 0:ARH].opt()], outs=[ar_out_a.opt()])
                        nc.sync.dma_start(
                            csb[:, :ARH // P],
                            ar_out_a.rearrange("o (a b) -> (o a) b", a=P))

                for ct in range(NCT):
                    acc = vs.tile([P, CW], bf16, tag="acc", name="acc")
                    for rc in range(NRC):
                        plog = plp.tile([P, CW], f32, tag="pl", name="plog")
                        for h in range(NH):
                            nc.tensor.matmul(
                                plog[:, h * 512:(h + 1) * 512],
                                itl[:, rc * P:(rc + 1) * P],
                                npf[:, ct * CW + h * 512: ct * CW + (h + 1) * 512])
                        e = st.tile([P, CW], bf16, tag="e", name="e", bufs=6)
                        slot = rc * NCT + ct
                        nc.scalar.activation(e[:], plog[:], AF.Exp,
                                             accum_out=rowacc[:, slot:slot + 1])
                        if rc == 0:
                            nc.vector.tensor_copy(acc[:], e[:])
                        else:
                            nc.vector.tensor_add(acc[:], acc[:], e[:])
                        if rc == 1 and pending is not None:
                            finalize_colsum(*pending)
                            pending = None
                    pending = (acc, ct)
                finalize_colsum(*pending)

                # ---- row direction partials ----
                rowsum = sb.tile([P, NRC], f32)
                nc.vector.reduce_sum(
                    rowsum[:],
                    rowacc.rearrange("p (rc ct) -> p rc ct", ct=NCT), axis=AX.X)
                lse_r = sb.tile([P, NRC], f32)
                nc.scalar.activation(lse_r[:], rowsum[:], AF.Ln)
                lsum = sb.tile([P, 1], f32)
                nc.vector.reduce_sum(lsum[:], lse_r[:], axis=AX.X)
                pR = plp.tile([P, CW], f32, tag="pl", name="pR")
                nc.tensor.matmul(pR[:1, :1], ones_kf[:], lsum[:])
                rpart = sb.tile([1, 1], f32)
                nc.vector.tensor_sub(rpart[:], pR[:1, :1], dsum[:])
                nc.sync.dma_start(ar_in[:1, b_total:b_total + 1], rpart[:])
                nc.sync.dma_start(ar_in[:1, b_total + 1:b_total + 2], dsum[:])

                # ---- AllReduce (second half + scalars) ----
                nc.gpsimd.collective_compute(
                    "AllReduce", Alu.add, replica_groups=rg,
                    ins=[ar_in[:1, ARH:ARW].opt()], outs=[ar_out_b.opt()])

                # ---- final ----
                # sum_j log(colsum_j) is order-independent, so any colsum
                # layout works.
                HB2 = b_total - ARH
                nc.sync.dma_start(
                    csb[:, ARH // P:],
                    ar_out_b[:1, :HB2].rearrange("o (a b) -> (o a) b", a=P))
                sc2 = sb.tile([1, 2], f32)
                nc.sync.dma_start(sc2[:], ar_out_b[:1, HB2:HB2 + 2])
                lse_c = sb.tile([P, b_total // P], f32)
                nc.scalar.activation(lse_c[:], csb[:], AF.Ln)
                csum_p = sb.tile([P, 1], f32)
                nc.vector.reduce_sum(csum_p[:], lse_c[:], axis=AX.X)
                pC = plp.tile([P, CW], f32, tag="pl", name="pC")
                nc.tensor.matmul(pC[:1, :1], ones_kf[:], csum_p[:])
                t1 = sb.tile([1, 1], f32)
                nc.vector.tensor_add(t1[:], pC[:1, :1], sc2[:1, 0:1])
                t2 = sb.tile([1, 1], f32)
                nc.vector.tensor_sub(t2[:], t1[:], sc2[:1, 1:2])
                lsb = sb.tile([1, 1], f32)
                nc.vector.tensor_scalar_mul(lsb[:], t2[:], 1.0 / (2.0 * b_total))
                nc.sync.dma_start(loss, lsb[:])

    nc.compile()
    _NC_CACHE[key] = nc
    return nc


def _bf16(x):
    """Cast f32 -> bfloat16 (round-to-nearest-even) host-side."""
    try:
        import ml_dtypes
        return np.asarray(x, dtype=np.float32).astype(ml_dtypes.bfloat16)
    except ImportError:
        f = np.ascontiguousarray(np.asarray(x, dtype=np.float32))
        u = f.view(np.uint32)
        r = ((u >> 16) & 1) + 0x7FFF
        return ((u + r) >> 16).astype(np.uint16)


def shard_inputs(inputs, b_total=B, n_cores=N_CORES):
    BL = b_total // n_cores
    img = np.asarray(inputs["img_feat"], dtype=np.float32)
    num = np.asarray(inputs["num_feat"], dtype=np.float32)

    def col(name):
        return np.ascontiguousarray(
            np.asarray(inputs[name], dtype=np.float32).reshape(P, 1))

    lt = np.asarray(inputs["log_temp"], dtype=np.float32).reshape(1, 1)
    shared = {
        "Wi1": _bf16(inputs["Wi1"]), "Wi2": _bf16(inputs["Wi2"]),
        "Wn1": _bf16(inputs["Wn1"]), "Wn2": _bf16(inputs["Wn2"]),
        "bi1": col("bi1"), "bi2": col("bi2"),
        "bn1": col("bn1"), "bn2": col("bn2"),
        "log_temp": np.ascontiguousarray(lt),
    }
    maps = []
    for c in range(n_cores):
        m = dict(shared)
        m["imgT"] = np.ascontiguousarray(_bf16(img[c * BL:(c + 1) * BL]).T)
        m["numT"] = np.ascontiguousarray(_bf16(num[c * BL:(c + 1) * BL]).T)
        maps.append(m)
    return maps


def run(inputs, trace=False, **kw):
    """Run on hardware; returns (loss_scalar, BassKernelResults)."""
    from concourse.bass_utils import run_bass_kernel_spmd
    nc = build()
    res = run_bass_kernel_spmd(nc, shard_inputs(inputs),
                               core_ids=list(range(N_CORES)), trace=trace, **kw)
    val = np.asarray(res.results[0]["loss"], dtype=np.float32).reshape(())
    return val, res


def kernel(**inputs):
    val, _ = run(inputs)
    return val
